# revision 1
# baseline (speedup 1.0000x reference)
"""Anisotropic diffusion step on 8 TRN2 NeuronCores.

Problem: x_new = x + sigmoid(rate) * cond * lap * (1 - mask)  (blend form)
  grad = Sobel(x); gm = sqrt(gx^2+gy^2+eps)
  cond = sigmoid(W2 @ relu(W1 @ [gm; x] + b1) + b2)   (1x1 convs)
  lap  = Laplacian(x)

Sharding: data-parallel over batch. B=8, one image (64,256,256) per core.
All params replicated. No collectives.

Per-core layout: partitions 0-63 = 64 channels of image-half A (rows
0..127), partitions 64-127 = channels of half B (rows 128..255).
x is stored once in SBUF as bf16 with 1-row halos and 1-column zero
padding (row stride 258).  Two super-blocks of 64 rows/half, each with a
sqrt phase (stencils -> gm) and a sigmoid phase (matmuls + update), so
the ScalarE activation-table set is switched only 4x.
"""

import numpy as np
from contextlib import ExitStack

C = 64
H = 256
W = 256
HID = 64
HH = 128          # rows per half
NR = 8            # rows per block (per half)
NBLK = H // 2 // NR   # 16 blocks total
NSB = 2           # super blocks
BLK_PER_SB = NBLK // NSB
WP = W + 2        # padded row stride in x sbuf tile
XROWS = HH + 2    # rows per half incl halos
EPS = 1e-8
USE_FP8_TAPS = True

_CACHE = {}


def _register_custom_ops():
    from concourse import dve_ops as DO
    if any(o.name == "ANT_SQSUM" for o in DO.OPS):
        return
    from concourse.dve_spec import Spec, Src0, Src1, C2, sq, lower, _has_src1
    from concourse.dve_uop import DveOpSpec
    body = sq(Src0) + sq(Src1) + C2
    spec = Spec(body=body,
                reference=lambda in0, in1, s0, s1, imm2: in0 * in0 + in1 * in1 + imm2)
    shas = {}
    for ver in ("v3", "v4"):
        tmp = DveOpSpec(name="ANT_SQSUM", uops=lower(spec, ver=ver),
                        rd1_en=_has_src1(spec))
        shas[ver] = tmp.sha(ver)
    op = DO.DveOp("ANT_SQSUM", spec, subdim=False, uops_sha=shas)
    DO.OPS.append(op)
    DO.CUSTOM_DVE_SPECS[op.name] = op.spec
    DO._SUB_OPCODE_FOR_NAME[op.name] = DO._CUSTOM_DVE_ROW_BASE + len(DO.OPS) - 1


def _build():
    import concourse.bass as bass
    import concourse.bacc as bacc
    import concourse.tile as tile
    from concourse import mybir
    ALU = mybir.AluOpType

    f32 = mybir.dt.float32
    bf16 = mybir.dt.bfloat16
    AF = mybir.ActivationFunctionType
    OP = mybir.AluOpType

    _register_custom_ops()
    from concourse.dve_ops import OPS as _DVE_OPS
    SQSUM = [o for o in _DVE_OPS if o.name == "ANT_SQSUM"][0]

    nc = bacc.Bacc()

    x_ext = nc.declare_dram_parameter("x", [C, H, W], bf16, isOutput=False)
    mask_ext = nc.declare_dram_parameter("mask", [H, W], f32, isOutput=False)
    # host-prepped weights: transposed / duplicated / block-diagonalized, bf16
    w1at_ext = nc.declare_dram_parameter("w1at", [128, 128], bf16, isOutput=False)
    w1bt_ext = nc.declare_dram_parameter("w1bt", [128, 128], bf16, isOutput=False)
    w2d_ext = nc.declare_dram_parameter("w2d", [128, 128], bf16, isOutput=False)
    b1d_ext = nc.declare_dram_parameter("b1d", [128, 1], f32, isOutput=False)
    b2d_ext = nc.declare_dram_parameter("b2d", [128, 1], f32, isOutput=False)
    lp_ext = nc.declare_dram_parameter("lp", [2, 128], bf16, isOutput=False)
    eyp1_ext = nc.declare_dram_parameter("eyp1", [128, 128], bf16, isOutput=False)
    eyp2_ext = nc.declare_dram_parameter("eyp2", [128, 128], bf16, isOutput=False)
    eym1_ext = nc.declare_dram_parameter("eym1", [128, 128], bf16, isOutput=False)
    eym2_ext = nc.declare_dram_parameter("eym2", [128, 128], bf16, isOutput=False)
    eym4_ext = nc.declare_dram_parameter("eym4", [128, 128], bf16, isOutput=False)
    f8 = mybir.dt.float8e4
    if USE_FP8_TAPS:
        xf8_ext = nc.declare_dram_parameter("xf8", [C, H, W], f8, isOutput=False)
        edr1_ext = nc.declare_dram_parameter("edr1", [128, 256], f8, isOutput=False)
        edr2_ext = nc.declare_dram_parameter("edr2", [128, 256], f8, isOutput=False)
        edrp_ext = nc.declare_dram_parameter("edrp", [128, 256], f8, isOutput=False)
        em4_ext = nc.declare_dram_parameter("em4", [128, 128], f8, isOutput=False)
    dr_ext = nc.declare_dram_parameter("dr", [1, 1], f32, isOutput=False)
    out_ext = nc.declare_dram_parameter("out", [C, H, W], bf16, isOutput=True)
    mp_dram = nc.dram_tensor("mp_dram", [2, (H // 2) * W], bf16)

    ctx = ExitStack()
    with TileCtx(nc, tile) as tc, ctx:
        persist = ctx.enter_context(tc.tile_pool(name="persist", bufs=1))
        wpool = ctx.enter_context(tc.tile_pool(name="wts", bufs=1))
        stage = ctx.enter_context(tc.tile_pool(name="stage", bufs=3))
        scratch = ctx.enter_context(tc.tile_pool(name="scr", bufs=1))
        sc2 = ctx.enter_context(tc.tile_pool(name="scr2", bufs=3))
        psum = ctx.enter_context(tc.tile_pool(name="ps", bufs=2, space="PSUM"))

        # ---------------- persistent tiles ----------------
        XB = persist.tile([128, XROWS * WP], bf16, tag="XB")     # padded x
        GM = persist.tile([128, (HH // NSB) * W], bf16, tag="GM")  # grad mag (per SB)

        xb = XB[:].rearrange("p (r c) -> p r c", r=XROWS)
        gm3 = GM[:].rearrange("p (r c) -> p r c", r=HH // NSB)

        # ---------------- weights ----------------
        W1A = wpool.tile([128, 128], bf16, tag="W1A")  # blockdiag(w1a^T, w1a^T)
        W1B = wpool.tile([128, 128], bf16, tag="W1B")  # blockdiag(w1b^T, w1b^T)
        W2D = wpool.tile([128, 128], bf16, tag="W2D")  # blockdiag(w2^T, w2^T)
        B1 = wpool.tile([128, 1], f32, tag="B1")
        B2 = wpool.tile([128, 1], f32, tag="B2")
        ONES = wpool.tile([128, HID], bf16, tag="ONES")
        R128 = wpool.tile([128, 1], f32, tag="R128")
        NR128 = wpool.tile([128, 1], f32, tag="NR128")
        MP128 = wpool.tile([128, 512], bf16, tag="MP128")  # r*(1-mask)
        LP = wpool.tile([2, 128], bf16, tag="LP")  # blockdiag ones for mask bcast
        EYP1 = wpool.tile([128, 128], bf16, tag="EYP1")
        EYP2 = wpool.tile([128, 128], bf16, tag="EYP2")
        EYM1 = wpool.tile([128, 128], bf16, tag="EYM1")
        EYM2 = wpool.tile([128, 128], bf16, tag="EYM2")
        EYM4 = wpool.tile([128, 128], bf16, tag="EYM4")
        if USE_FP8_TAPS:
            XF8 = persist.tile([128, XROWS * WP], f8, tag="XF8")
            xf8 = XF8[:].rearrange("p (r c) -> p r c", r=XROWS)
            EDR1 = wpool.tile([128, 256], f8, tag="EDR1")
            EDR2 = wpool.tile([128, 256], f8, tag="EDR2")
            EDRP = wpool.tile([128, 256], f8, tag="EDRP")
            EM4 = wpool.tile([128, 128], f8, tag="EM4")
        DRS = wpool.tile([1, 1], f32, tag="DRS")
        RSG = wpool.tile([1, 1], f32, tag="RSG")
        EPSB = wpool.tile([128, 1], f32, tag="EPSB")
        ZB = wpool.tile([128, 1], f32, tag="ZB")
        nc.vector.memset(EPSB[:, :], EPS)
        nc.vector.memset(ZB[:, :], 0.0)

        nc.sync.dma_start(out=W1A[:, :], in_=w1at_ext[:, :])
        nc.sync.dma_start(out=W1B[:, :], in_=w1bt_ext[:, :])
        nc.sync.dma_start(out=W2D[:, :], in_=w2d_ext[:, :])
        nc.sync.dma_start(out=B1[:, :], in_=b1d_ext[:, :])
        nc.sync.dma_start(out=B2[:, :], in_=b2d_ext[:, :])
        nc.sync.dma_start(out=LP[:, :], in_=lp_ext[:, :])
        nc.sync.dma_start(out=EYP1[:, :], in_=eyp1_ext[:, :])
        nc.sync.dma_start(out=EYP2[:, :], in_=eyp2_ext[:, :])
        nc.sync.dma_start(out=EYM1[:, :], in_=eym1_ext[:, :])
        nc.sync.dma_start(out=EYM2[:, :], in_=eym2_ext[:, :])
        nc.sync.dma_start(out=EYM4[:, :], in_=eym4_ext[:, :])
        if USE_FP8_TAPS:
            nc.sync.dma_start(out=EDR1[:, :], in_=edr1_ext[:, :])
            nc.sync.dma_start(out=EDR2[:, :], in_=edr2_ext[:, :])
            nc.sync.dma_start(out=EDRP[:, :], in_=edrp_ext[:, :])
            nc.sync.dma_start(out=EM4[:, :], in_=em4_ext[:, :])
        nc.vector.memset(ONES[:, :], 1.0)
        nc.sync.dma_start(out=DRS[:, :], in_=dr_ext[:, :])

        tc.strict_bb_all_engine_barrier()

        # ---------------- x padding memsets ----------------
        nc.vector.memset(xb[:, :, 0:1], 0.0)
        nc.vector.memset(xb[:, :, WP - 1:WP], 0.0)
        nc.vector.memset(xb[0:64, 0:1, :], 0.0)           # half A row -1
        nc.vector.memset(xb[64:128, XROWS - 1:XROWS, :], 0.0)  # half B row 256

        # cross-half halo rows: A idx 129 <- row 128 ; B idx 0 <- row 127
        nc.sync.dma_start(out=xb[0:64, XROWS - 1, 1:W + 1], in_=x_ext[:, HH, :])
        nc.sync.dma_start(out=xb[64:128, 0, 1:W + 1], in_=x_ext[:, HH - 1, :])
        if USE_FP8_TAPS:
            nc.vector.memset(xf8[:, :, 0:1], 0.0)
            nc.vector.memset(xf8[:, :, WP - 1:WP], 0.0)
            nc.vector.memset(xf8[0:64, 0:1, :], 0.0)
            nc.vector.memset(xf8[64:128, XROWS - 1:XROWS, :], 0.0)
            nc.sync.dma_start(out=xf8[0:64, XROWS - 1, 1:W + 1], in_=xf8_ext[:, HH, :])
            nc.sync.dma_start(out=xf8[64:128, 0, 1:W + 1], in_=xf8_ext[:, HH - 1, :])

        import bass_rust as _br

        def _pair_view(tile, row_idx, pair_stride, col0):
            """(128, 2, 2, 256) strided view of a padded (XROWS*WP) tile."""
            c = tile[:].copy()
            c.ap = _br.VecI64Pair(
                [(XROWS * WP, 128), (pair_stride, 2), (WP, 2), (1, 256)])
            c.offset = row_idx * WP + col0
            return c

        def load_block(g):
            """DMA rows [g*NR, g*NR+NR) of each half straight into XB."""
            r0 = g * NR
            nc.sync.dma_start(out=xb[0:64, r0 + 1:r0 + 1 + NR, 1:W + 1],
                              in_=x_ext[:, r0:r0 + NR, :])
            nc.sync.dma_start(out=xb[64:128, r0 + 1:r0 + 1 + NR, 1:W + 1],
                              in_=x_ext[:, HH + r0:HH + r0 + NR, :])
            if USE_FP8_TAPS:
                nc.sync.dma_start(out=xf8[0:64, r0 + 1:r0 + 1 + NR, 1:W + 1],
                                  in_=xf8_ext[:, r0:r0 + NR, :])
                nc.sync.dma_start(out=xf8[64:128, r0 + 1:r0 + 1 + NR, 1:W + 1],
                                  in_=xf8_ext[:, HH + r0:HH + r0 + NR, :])

        def stencil_block(g, sb):
            """Sobel grad magnitude for rows [g*NR, g*NR+NR) of each half.

            gx on DVE (separable, shares P); gy as 6 shifted-diagonal
            matmuls on TensorE; g2 = gx^2+gy^2+eps fused in one custom DVE
            op reading gy straight from PSUM; sqrt on ScalarE."""
            r0 = g * NR
            lr0 = r0 - sb * (HH // NSB)
            nw = NR + 2
            G2 = scratch.tile([128, NR * W], bf16, tag="t4")
            nchunk = NR * W // 512
            rpc = 512 // W
            if not USE_FP8_TAPS:
                xw = xb[:, r0:r0 + nw, :]      # rows r0-1 .. r0+NR in img space
                P = scratch.tile([128, nw * W], bf16, tag="t0")
                p3 = P[:].rearrange("p (r c) -> p r c", r=nw)
                nc.vector.tensor_tensor(p3[:, :, :], xw[:, :, 2:W + 2], xw[:, :, 0:W], ALU.subtract)
                GXA = scratch.tile([128, NR * W], bf16, tag="t5")
                P2 = scratch.tile([128, NR * W], bf16, tag="t6")
                GX = scratch.tile([128, NR * W], bf16, tag="t7")
                gxa3 = GXA[:].rearrange("p (r c) -> p r c", r=NR)
                p23 = P2[:].rearrange("p (r c) -> p r c", r=NR)
                gx3 = GX[:].rearrange("p (r c) -> p r c", r=NR)
                nc.vector.tensor_tensor(gxa3[:], p3[:, 2:nw, :], p3[:, 0:nw - 2, :], ALU.add)
                nc.vector.tensor_scalar_mul(p23[:], p3[:, 1:nw - 1, :], 2.0)
                nc.vector.tensor_tensor(gx3[:], gxa3[:], p23[:], ALU.add)
            for cidx in range(nchunk):
                ra = r0 + cidx * rpc
                rot = ["stps", "hps", "zps", "mps"]
                k2 = (g * nchunk + cidx) * 2
                gyp = psum.tile([128, 512], mybir.dt.float32, tag=rot[k2 % 4])
                if USE_FP8_TAPS:
                    # sobel_y: rows (r-1, r+1) paired per DoubleRow matmul
                    for ti, (dc, edr) in enumerate([(-1, EDR1), (0, EDR2), (1, EDR1)]):
                        nc.tensor.matmul(
                            gyp[:, :],
                            edr[:].rearrange("p (a m) -> p a m", a=2),
                            _pair_view(XF8, ra, 2 * WP, 1 + dc),
                            start=(ti == 0), stop=(ti == 2),
                            perf_mode=mybir.MatmulPerfMode.DoubleRow)
                    GYS = sc2.tile([128, 512], bf16, tag="gys")
                    nc.scalar.activation(GYS[:, :], gyp[:, :], AF.Copy)
                    # sobel_x: cols (c-1, c+1) paired per DoubleRow matmul
                    gxp = psum.tile([128, 512], mybir.dt.float32, tag=rot[(k2 + 1) % 4])
                    for ti, (dr, edr) in enumerate([(-1, EDR1), (0, EDR2), (1, EDR1)]):
                        nc.tensor.matmul(
                            gxp[:, :],
                            edr[:].rearrange("p (a m) -> p a m", a=2),
                            _pair_view(XF8, ra + 1 + dr, 2, 0),
                            start=(ti == 0), stop=(ti == 2),
                            perf_mode=mybir.MatmulPerfMode.DoubleRow)
                    nc.vector._custom_dve(
                        SQSUM, out=G2[:, cidx * 512:(cidx + 1) * 512],
                        in0=gxp[:, :], in1=GYS[:, :], imm2=EPS)
                else:
                    GY_TAPS = [(1, -1, EYP1), (1, 0, EYP2), (1, 1, EYP1),
                               (-1, -1, EYM1), (-1, 0, EYM2), (-1, 1, EYM1)]
                    for ti, (dr, dc, eye) in enumerate(GY_TAPS):
                        nc.tensor.matmul(
                            gyp[:, :], eye[:, :],
                            xb[:, ra + 1 + dr:ra + 1 + dr + rpc, 1 + dc:1 + dc + W],
                            start=(ti == 0), stop=(ti == len(GY_TAPS) - 1))
                    nc.vector._custom_dve(
                        SQSUM, out=G2[:, cidx * 512:(cidx + 1) * 512],
                        in0=GX[:, cidx * 512:(cidx + 1) * 512], in1=gyp[:, :],
                        imm2=EPS)
            # gm = sqrt(g2)
            nc.scalar.activation(gm3[:, lr0:lr0 + NR, :],
                                 G2[:].rearrange("p (r c) -> p r c", r=NR),
                                 AF.Sqrt, bias=ZB[:, :])

        def phase2_block(g, sb):
            """Matmuls + Laplacian + update for rows [g*NR, ...) of each half."""
            r0 = g * NR
            lr0 = r0 - sb * (HH // NSB)
            MPB = stage.tile([2, NR * W], bf16, tag="mpb")
            nc.sync.dma_start(out=MPB[:, :], in_=mp_dram[:, r0 * W:(r0 + NR) * W])
            XO = stage.tile([128, NR * W], bf16, tag="xo")
            xo3 = XO[:].rearrange("p (r c) -> p r c", r=NR)
            nchunk = NR * W // 512
            rows_per_chunk = 512 // W
            for cidx in range(nchunk):
                ra = r0 + cidx * rows_per_chunk     # absolute row in half
                la = lr0 + cidx * rows_per_chunk    # row in GM tile
                n0 = ra * W                          # pixel offset in half
                pA = n0 // 512
                pB = 64 + pA
                hps = psum.tile([128, 512], mybir.dt.float32, tag="hps")
                nc.tensor.matmul(
                    hps[:, :], W1A[:, :],
                    gm3[:, la:la + rows_per_chunk, :],
                    start=True, stop=False)
                nc.tensor.matmul(
                    hps[:, :], W1B[:, :],
                    xb[:, ra + 1:ra + 1 + rows_per_chunk, 1:W + 1],
                    start=False, stop=True)
                HR = sc2.tile([128, 512], bf16, tag="hr")
                nc.scalar.activation(HR[:, :], hps[:, :], AF.Relu, bias=B1[:, :])
                zps = psum.tile([128, 512], mybir.dt.float32, tag="zps")
                nc.tensor.matmul(zps[:, :], W2D[:, :], HR[:, :], start=True, stop=True)
                CD = sc2.tile([128, 512], bf16, tag="cd")
                nc.scalar.activation(CD[:, :], zps[:, :], AF.Sigmoid, bias=B2[:, :])
                mps = psum.tile([128, 512], mybir.dt.float32, tag="mps")
                nc.tensor.matmul(mps[:, :], LP[:, :],
                                 MPB[:, cidx * 512:(cidx + 1) * 512],
                                 start=True, stop=True)
                # Laplacian on TensorE (bf16: it feeds the update linearly,
                # fp8 here costs ~2.8e-2 rel err)
                lps = psum.tile([128, 512], mybir.dt.float32, tag="stps")
                LAP_TAPS = [(-1, 0, EYP1), (1, 0, EYP1), (0, -1, EYP1),
                            (0, 1, EYP1), (0, 0, EYM4)]
                for ti, (dr, dc, eye) in enumerate(LAP_TAPS):
                    nc.tensor.matmul(
                        lps[:, :], eye[:, :],
                        xb[:, ra + 1 + dr:ra + 1 + dr + rows_per_chunk,
                           1 + dc:1 + dc + W],
                        start=(ti == 0), stop=(ti == len(LAP_TAPS) - 1))
                CL = sc2.tile([128, 512], bf16, tag="cl")
                nc.vector.tensor_tensor(CL[:, :], CD[:, :], lps[:, :], ALU.mult)
                UU = sc2.tile([128, 512], bf16, tag="uu")
                nc.vector.tensor_tensor(UU[:, :], CL[:, :], mps[:, :], ALU.mult)
                xcv = xb[:, ra + 1:ra + 1 + rows_per_chunk, 1:W + 1]
                nc.vector.tensor_tensor(
                    xo3[:, cidx * rows_per_chunk:(cidx + 1) * rows_per_chunk, :],
                    xcv, UU[:].rearrange("p (r c) -> p r c", r=2), ALU.add)
            nc.sync.dma_start(out=out_ext[:, r0:r0 + NR, :], in_=xo3[0:64])
            nc.sync.dma_start(out=out_ext[:, HH + r0:HH + r0 + NR, :], in_=xo3[64:128])

        # ================= main schedule =================
        for sb in range(NSB):
            # ---- phase 1: loads + stencils + sqrt ----
            if sb == 0:
                for g in range(0, BLK_PER_SB + 1):
                    load_block(g)
                    if g >= 1:
                        stencil_block(g - 1, sb)
            else:
                for g in range(BLK_PER_SB + 1, NBLK + 1):
                    if g < NBLK:
                        load_block(g)
                    stencil_block(g - 1, sb)
            tc.no_sync_barrier()
            # ---- phase 2 ----
            if sb == 0:
                # r = sigmoid(diffusion_rate), broadcast, mp = r*(1-mask)
                nc.scalar.activation(RSG[:, :], DRS[:, :], AF.Sigmoid, bias=ZB[0:1, :])
                ones_f = wpool.tile([1, 128], f32, tag="onesf")
                nc.vector.memset(ones_f[:, :], 1.0)
                rps = psum.tile([128, 1], mybir.dt.float32, tag="stps")
                nc.tensor.matmul(rps[:, :], ones_f[:, :], RSG[:, :], start=True, stop=True)
                nc.scalar.activation(R128[:, :], rps[:, :], AF.Copy)
                nc.scalar.activation(NR128[:, :], rps[:, :], AF.Copy, scale=-1.0)
                MK = wpool.tile([128, 512], f32, tag="MK")
                mk3 = MK[:].rearrange("p (r c) -> p r c", r=512 // W)
                nc.sync.dma_start(
                    out=mk3[:, :, :],
                    in_=mask_ext[:].rearrange("(p r) c -> p r c", p=128))
                nc.scalar.activation(MP128[:, :], MK[:, :], AF.Identity,
                                     bias=R128[:, :], scale=NR128[:, :])
                nc.sync.dma_start(
                    out=mp_dram[:].rearrange("h (g n) -> (h g) n", g=64),
                    in_=MP128[:, :])
            for g in range(sb * BLK_PER_SB, (sb + 1) * BLK_PER_SB):
                phase2_block(g, sb)
            tc.no_sync_barrier()

    nc.compile()
    return nc


def TileCtx(nc, tile):
    return tile.TileContext(nc)


def _get_nc():
    if "nc" not in _CACHE:
        _CACHE["nc"] = _build()
    return _CACHE["nc"]


def _run(inputs, trace=False):
    from concourse.bass_utils import run_bass_kernel_spmd

    import ml_dtypes

    nc = _get_nc()
    bf = ml_dtypes.bfloat16
    x = np.asarray(inputs["x"], dtype=np.float32)
    mask = np.asarray(inputs["mask"], dtype=np.float32)
    w1 = np.asarray(inputs["w1"], dtype=np.float32)
    b1 = np.asarray(inputs["b1"], dtype=np.float32).reshape(HID, 1)
    w2 = np.asarray(inputs["w2"], dtype=np.float32)
    b2 = np.asarray(inputs["b2"], dtype=np.float32).reshape(C, 1)
    dr = np.asarray(inputs["diffusion_rate"], dtype=np.float32).reshape(1, 1)

    from concourse import mybir as _mb0
    f8np = _mb0.dt.np(_mb0.dt.float8e4)
    w1at = np.zeros((128, 128), dtype=np.float32)
    w1at[0:C, 0:HID] = w1[:, 0:C].T
    w1at[C:128, HID:128] = w1[:, 0:C].T
    w1at = w1at.astype(bf)
    w1bt = np.zeros((128, 128), dtype=np.float32)
    w1bt[0:C, 0:HID] = w1[:, C:2 * C].T
    w1bt[C:128, HID:128] = w1[:, C:2 * C].T
    w1bt = w1bt.astype(bf)
    w2d = np.zeros((128, 128), dtype=np.float32)
    w2d[0:HID, 0:C] = w2.T
    w2d[HID:128, C:128] = w2.T
    w2d = w2d.astype(bf)
    b1d = np.concatenate([b1, b1], axis=0)
    b2d = np.concatenate([b2, b2], axis=0)
    lp = np.zeros((2, 128), dtype=np.float32)
    lp[0, 0:64] = 1.0
    lp[1, 64:128] = 1.0
    lp = lp.astype(bf)
    eye = np.eye(128, dtype=np.float32)
    eyp1 = (eye * 1.0).astype(bf)
    eyp2 = (eye * 2.0).astype(bf)
    eym1 = (eye * -1.0).astype(bf)
    eym2 = (eye * -2.0).astype(bf)
    eym4 = (eye * -4.0).astype(bf)
    def _dr(w0, w1):
        a = np.zeros((128, 2, 128), dtype=np.float32)
        a[:, 0, :] = eye * w0
        a[:, 1, :] = eye * w1
        return np.ascontiguousarray(a.reshape(128, 256)).astype(f8np)
    edr1 = _dr(-1.0, 1.0)
    edr2 = _dr(-2.0, 2.0)
    edrp = _dr(1.0, 1.0)
    em4 = (eye * -4.0).astype(f8np)

    xbf = np.ascontiguousarray(x.astype(bf))
    xf8a = np.ascontiguousarray(x.astype(f8np))
    B = x.shape[0]
    in_maps = []
    for b in range(B):
        in_maps.append({
            "x": xbf[b],
            "mask": np.ascontiguousarray(mask[b, 0]),
            "w1at": w1at, "w1bt": w1bt, "w2d": w2d,
            "b1d": b1d, "b2d": b2d, "lp": lp, "dr": dr,
            "eyp1": eyp1, "eyp2": eyp2, "eym1": eym1,
            "eym2": eym2, "eym4": eym4,
            "xf8": xf8a[b], "edr1": edr1, "edr2": edr2,
            "edrp": edrp, "em4": em4,
        })
    res = run_bass_kernel_spmd(nc, in_maps, core_ids=list(range(8)), trace=trace)
    out = np.stack([np.asarray(res.results[i]["out"]).astype(np.float32)
                    for i in range(B)], axis=0)
    return out, res.exec_time_ns


def kernel(**inputs):
    return _run(inputs, trace=False)[0]

